# revision 20
# baseline (speedup 1.0000x reference)
"""Trainium2 Bass kernel for nn_ConnectLossV2 (BCE+Dice connectivity loss).

Strategy (8 cores, data-parallel over pixels):
  - Shard the B*H*W = 2,359,296 pixels as (batch b = core//2, H-half = core%2),
    294,912 pixels per core.
  - Per core, everything reduces to a 17x55 matrix of segment sums
      S[n, c] = sum_{pixels p: target[p]==n} payload_c[p]
    where the 55 payload columns are, for 18 "channels" (pred ch 0..16, cls):
      raw p (18) | log(max(p,EPS)) (18) | log1p(-p) (18) | ones (1).
    Computed as one-hot matmuls accumulated in PSUM:
      S += onehot(tm)[128px, 17].T @ payload[128px, 55]
    using 4-way tensor-engine column tiling (4 independent 17-col matmuls
    in flight in different 32-column groups of the PE array).
  - Host sums the per-core / per-column-group partials in float64 and
    assembles BCE/Dice terms + the tiny 16x16 greedy matching.
"""

import sys

sys.path.insert(0, "/opt/trn_rl_repo")

import numpy as np

EPS = 1e-7
N_INST = 16
P = 128          # SBUF partitions / matmul contraction
F = 256          # pixels per f-chunk per lane
NCHUNK = 9       # 9 * F = 2304 pixels per lane
NCH = 18         # payload channels: pred 0..16, cls
NSEG = 17        # target ids 0..16
NPAY = 3 * NCH + 1  # 55: raw | logp | log1mp | ones
NG = 4           # PE column-tiling groups
NCORES = 8

_compiled = None


def _build(reps=1, do_onehot=True, do_logs=True, do_mm=True, do_dma=True,
           mm_stride=1, bufs=2, ng=NG):
    import concourse.bacc as bacc
    import concourse.tile as tile
    from concourse import mybir

    nc = bacc.Bacc("TRN2", target_bir_lowering=False, debug=False,
                   num_devices=NCORES)

    pred_in = nc.dram_tensor("pred", [17, 384, 768], mybir.dt.float32,
                             kind="ExternalInput").ap()
    cls_in = nc.dram_tensor("cls", [384, 768], mybir.dt.float32,
                            kind="ExternalInput").ap()
    tm_in = nc.dram_tensor("tm", [384, 768], mybir.dt.int32,
                           kind="ExternalInput").ap()
    s_out = nc.dram_tensor("s", [P, NPAY], mybir.dt.float32,
                           kind="ExternalOutput").ap()

    # lane l <-> 3 consecutive image rows; free dim = 2304 pixels per lane
    pred_r = pred_in.rearrange("k (l r) w -> l k (r w)", r=3)   # [128,17,2304]
    cls_r = cls_in.rearrange("(l r) w -> l (r w)", r=3)         # [128,2304]
    tm_r = tm_in.rearrange("(l r) w -> l (r w)", r=3)           # [128,2304]

    bf16 = mybir.dt.bfloat16
    with tile.TileContext(nc) as tc:
        with (
            tc.tile_pool(name="raw", bufs=bufs) as raw_pool,
            tc.tile_pool(name="pay", bufs=bufs) as pay_pool,
            tc.tile_pool(name="oh", bufs=bufs) as oh_pool,
            tc.tile_pool(name="tmp", bufs=bufs) as tmp_pool,
            tc.tile_pool(name="fin", bufs=1) as fin_pool,
            tc.tile_pool(name="ps", bufs=1, space="PSUM") as ps_pool,
        ):
            psum = ps_pool.tile([P, NPAY], mybir.dt.float32)

            for rep in range(reps):
                for j in range(NCHUNK):
                    raw = raw_pool.tile([P, NCH, F], mybir.dt.float32,
                                        tag="raw")
                    pay = pay_pool.tile([P, NPAY, F], bf16, tag="pay")
                    oh = oh_pool.tile([P, NSEG, F], bf16, tag="oh")
                    tmi = tmp_pool.tile([P, F], mybir.dt.int32, tag="tmi")
                    tmf = tmp_pool.tile([P, F], bf16, tag="tmf")

                    fl, fh = j * F, (j + 1) * F
                    if do_dma:
                        nc.sync.dma_start(out=raw[:, 0:9, :],
                                          in_=pred_r[:, 0:9, fl:fh])
                        nc.sync.dma_start(out=raw[:, 9:17, :],
                                          in_=pred_r[:, 9:17, fl:fh])
                        nc.sync.dma_start(out=raw[:, 17, :],
                                          in_=cls_r[:, fl:fh])
                        nc.sync.dma_start(out=tmi[:], in_=tm_r[:, fl:fh])
                    else:
                        nc.vector.memset(raw[:, 0:1, 0:1], 0.5)
                        nc.vector.memset(tmi[:, 0:1], 1)
                    nc.vector.tensor_copy(tmf[:], tmi[:])

                    # one-hot of target ids (bf16, exact 0/1)
                    if do_onehot:
                        for n in range(NSEG):
                            nc.vector.tensor_scalar(
                                oh[:, n, :], tmf[:], float(n), None,
                                mybir.AluOpType.is_equal)

                    # payload: clip(p) | Ln(clip(p)) | Ln(1-p) | ones.  The
                    # "raw" block uses clipped p too: it only differs for
                    # p < EPS, which is negligible in the dice sums.
                    # Split into channel halves for finer pipelining.
                    for (a, b) in ((0, 9), (9, NCH)):
                        nc.vector.tensor_scalar(
                            pay[:, a:b, :], raw[:, a:b, :], EPS,
                            None, mybir.AluOpType.max)
                        if do_logs:
                            nc.scalar.activation(
                                pay[:, NCH + a:NCH + b, :], pay[:, a:b, :],
                                mybir.ActivationFunctionType.Ln)
                            nc.scalar.activation(
                                pay[:, 2 * NCH + a:2 * NCH + b, :],
                                raw[:, a:b, :],
                                mybir.ActivationFunctionType.Ln, bias=1.0,
                                scale=-1.0)
                    nc.vector.memset(pay[:, NPAY - 1, :], 1.0)

                    if do_mm:
                        for f in range(0, F, mm_stride):
                            g = (f // mm_stride) % ng
                            nc.tensor.matmul(
                                psum[32 * g:32 * g + NSEG, :],
                                oh[:, :, f] if do_onehot else pay[:, 0:NSEG, f],
                                pay[:, :, f],
                                start=(rep == 0 and j == 0
                                       and f < ng * mm_stride),
                                stop=(rep == reps - 1 and j == NCHUNK - 1
                                      and f >= F - ng * mm_stride),
                                tile_position=(None if ng == 1
                                               else (0, 32 * g)),
                                skip_group_check=True,
                            )
                    else:
                        # cheap consumers so loads/compute aren't dead
                        nc.vector.tensor_add(
                            psum[0:P, 0:1], tmf[:, 0:1], tmf[:, 0:1])
                        nc.vector.tensor_copy(psum[0:P, 1:2], pay[:, 0, 0:1])
                        if do_onehot:
                            nc.vector.tensor_copy(psum[0:P, 2:3],
                                                  oh[:, 0, 0:1])

            fin = fin_pool.tile([P, NPAY], mybir.dt.float32)
            nc.vector.tensor_copy(fin[:], psum[:])
            nc.sync.dma_start(out=s_out[:], in_=fin[:])

    nc.compile()
    return nc


def _get_compiled():
    global _compiled
    if _compiled is None:
        _compiled = _build()
    return _compiled


def _run_device(pred, cls_o, tm):
    """Run the per-core kernels; return S summed over cores/groups, float64 [17,55]."""
    from concourse.bass_utils import run_bass_kernel_spmd

    nc = _get_compiled()
    in_maps = []
    for c in range(NCORES):
        b, h0 = c // 2, (c % 2) * 384
        in_maps.append({
            "pred": np.ascontiguousarray(pred[b, :, h0:h0 + 384, :]),
            "cls": np.ascontiguousarray(cls_o[b, 0, h0:h0 + 384, :]),
            "tm": np.ascontiguousarray(tm[b, 0, h0:h0 + 384, :]),
        })
    res = run_bass_kernel_spmd(nc, in_maps, list(range(NCORES)))
    S = np.zeros((NSEG, NPAY), np.float64)
    for c in range(NCORES):
        s = res.results[c]["s"].astype(np.float64)
        for g in range(NG):
            S += s[32 * g:32 * g + NSEG, :]
    return S


def _assemble(S):
    """Host-side assembly of the final scalar loss from segment sums."""
    M = float(4 * 768 * 768)
    tot = S.sum(axis=0)                      # totals over all pixels, per payload col
    raw, logp, log1mp = S[:, 0:NCH], S[:, NCH:2 * NCH], S[:, 2 * NCH:3 * NCH]
    cnt = S[:, NPAY - 1]                     # [17] pixel count per target id
    t_raw, t_logp, t_log1mp = (tot[0:NCH], tot[NCH:2 * NCH],
                               tot[2 * NCH:3 * NCH])

    # term 1: cls_out (channel 17) vs tfg = (tm > 0)
    bce1 = -((t_logp[17] - logp[0, 17]) + log1mp[0, 17]) / M
    inter1 = t_raw[17] - raw[0, 17]
    dice1 = 1.0 - (2.0 * inter1 + EPS) / (t_raw[17] + (M - cnt[0]) + EPS)

    # term 2: pred channel 0 vs (1 - tfg)
    bce0 = -(logp[0, 0] + (t_log1mp[0] - log1mp[0, 0])) / M
    inter0 = raw[0, 0]
    dice0 = 1.0 - (2.0 * inter0 + EPS) / (t_raw[0] + cnt[0] + EPS)

    res = (bce1 + dice1) + (bce0 + dice0)

    # pairwise matrix L[n, k], n = 1..16 target ids, k = 1..16 pred channels
    k = np.arange(1, 17)
    A = -t_log1mp[k] / M                                     # [16]
    segD = log1mp[1:, :][:, k] - logp[1:, :][:, k]           # [16,16]
    segP = raw[1:, :][:, k]                                  # [16,16]
    bce = A[None, :] + segD / M
    dice = 1.0 - (2.0 * segP + EPS) / (t_raw[k][None, :] + cnt[1:, None] + EPS)
    L = (bce + dice).astype(np.float32)

    # greedy assignment
    avail = np.ones(16, bool)
    total = np.float32(0.0)
    for n in range(16):
        masked = np.where(avail, L[n], np.inf).astype(np.float32)
        i = int(np.argmin(masked))
        avail[i] = False
        total = np.float32(total + masked[i])
    return np.float32((np.float32(res) + total) / N_INST)


def kernel(pred_instance_mask, cls_out, target_mask):
    S = _run_device(np.asarray(pred_instance_mask), np.asarray(cls_out),
                    np.asarray(target_mask))
    return _assemble(S)
